# revision 65
# baseline (speedup 1.0000x reference)
"""AR(24) extrapolation kernel for Trainium2 (8 NeuronCores, data parallel).

The reference runs a 168-step scalar-weight autoregressive recurrence over the
last 24 timesteps of x, independently per (batch, channel).  Because the
recurrence is linear, output step t is a fixed linear combination of the
initial 24-sample window plus a bias term:

    y[b, t, d] = sum_i C[i, t] * x[b, S-24+i, d] + beta[t]

C [24, 168] and beta [168] follow from W/b by unrolling the recurrence once on
the host (float64, ~4k flops).  The device work is then a memory-bound
broadcast matmul: per core, out[t, (b, d)] = CB^T @ xaug where CB stacks
[C; beta] and xaug stacks [x_window^T; ones].

Sharding: pure data parallelism over batch (32 batches per core).

Layout details per core (all chosen so every DMA uses all 16 SBUF ports with
multi-KB contiguous runs, since the kernel is store-bandwidth-bound):
- input xpack [128, 4096]: 32 moving blocks of [25, 512] (24 window rows plus
  a ones row that carries the bias through the contraction); block b sits at
  row strip 32*(b%4) (PE operands must start on a 32-partition boundary) and
  columns (b//4)*512.
- weights cb [128, 168]: CB replicated into each 32-row strip.
- t chunk 0 (t 0..127): matmul stationary CB[:, :128] -> PSUM [128, 512] per
  batch; staged as [128, sz*512] and stored to out[128, 16384] ([t, b*D+d]).
- t tail (t 128..167): computed in transposed orientation (stationary =
  x d-chunk [25, 128], moving = CB[:, 128:168]) so the result lands as
  [d%128, t'] on all 128 partitions; staged [128, sz*160] and stored to
  outt[128, NB*4*40]; the host untransposes while gathering.
- float32r operands: fast fp32 matmul path on the PE (4x fp32 rate; ~1e-4
  relative error from TF32-style operand rounding).
- stores alternate between the two HWDGE rings (sync / scalar engines);
  group sizes [4, 4, 8, 8, 8] start the store stream early, which paces the
  whole kernel (~420 GB/s sustained, near the 435 GB/s SBUF fabric ceiling).
"""

import numpy as np

import concourse.bacc as bacc
import concourse.tile as tile
from concourse import mybir
from concourse.bass_utils import run_bass_kernel_spmd

ORDER = 24
K = ORDER + 1            # contraction: 24 window rows + ones row
T = 168
D = 512
B = 256
S = 336
N_CORES = 8
NB = B // N_CORES        # 32 local batches per core
COLS = NB * D            # 16384 columns per core
GROUPS = [4, 4, 8, 8, 8]  # batches per staged output group (small first so
                          # the store stream starts early, then steady-state)
assert sum(GROUPS) == NB
# store-chunk (bs, sub) list: first group stores in 2-batch chunks so the
# store stream starts as early as possible
SUBS = []
_b = 0
for _g, _sz in enumerate(GROUPS):
    for _s in ([2, 2] if _g == 0 else [_sz]):
        SUBS.append((_b, _s))
        _b += _s
F32 = mybir.dt.float32
F32R = mybir.dt.float32r  # fast fp32 matmul path (full PE rate at N>=256)

_nc_cache = None


def _build_program():
    nc = bacc.Bacc()
    xp = nc.declare_dram_parameter("xpack", [128, (NB // 4) * D], F32R, isOutput=False)
    cb = nc.declare_dram_parameter("cb", [128, T], F32R, isOutput=False)
    # chunk0: t 0..127 as [t, (b, d)]; tail: t 128..167 stored transposed as
    # [d%128, (g, j, d//128, t-128)] so its stores cover all 128 partitions
    out = nc.declare_dram_parameter("out", [128, COLS], F32, isOutput=True)
    outt = nc.declare_dram_parameter(
        "outt", [128, NB * 4 * (T - 128)], F32, isOutput=True
    )

    with tile.TileContext(nc) as tc:
        with (
            tc.tile_pool(name="consts", bufs=1) as consts,
            tc.tile_pool(name="xin", bufs=1) as xin,
            tc.tile_pool(name="stage", bufs=3) as stage,
            tc.tile_pool(name="psum", bufs=4, space="PSUM") as psum,
        ):
            # Early-needed input loads go on the HWDGE rings (idle until the
            # store stream starts): weights first (every matmul needs them),
            # and the first tile split by partition halves so batches 0-1
            # (rows 0-63) land before batches 2-3.  Late-group inputs ride
            # the gpsimd SWDGE ring, keeping the HWDGE rings clear so the
            # first stores are not queued behind input transfers.
            cb_t = consts.tile([128, T], F32R)
            nc.sync.dma_start(out=cb_t, in_=cb[:, :])
            starts = [sum(GROUPS[:g]) for g in range(len(GROUPS))]
            xts = []
            for g, (b0, sz) in enumerate(zip(starts, GROUPS)):
                xt = xin.tile([128, sz * 128], F32R, tag=f"xt{g}")
                src = xp[:, b0 * 128 : (b0 + sz) * 128]
                if g == 0:
                    nc.scalar.dma_start(out=xt[0:64, :], in_=src[0:64, :])
                    nc.scalar.dma_start(out=xt[64:128, :], in_=src[64:128, :])
                elif g == 1:
                    nc.sync.dma_start(out=xt, in_=src)
                else:
                    eng = nc.scalar if g % 2 == 0 else nc.sync
                    eng.dma_start(out=xt, in_=src)
                xts.append(xt)

            P0 = 128
            P1 = T - P0  # 40
            SPLITS = [[s for bs, s in SUBS if b0 <= bs < b0 + sz]
                      for b0, sz in zip(starts, GROUPS)]
            nstore = 0
            for g, (b0, sz) in enumerate(zip(starts, GROUPS)):
                sub0 = 0
                for sub in SPLITS[g]:
                    st0 = stage.tile([P0, sub * D], F32, tag="st0")
                    st1 = stage.tile([P0, sub * 4 * P1], F32, tag="st1")
                    # phase A: chunk0 matmuls (consecutive batches sit in
                    # different 32-row strips, so LDW overlaps the prior MM)
                    for jj in range(sub):
                        j = sub0 + jj
                        rs = 32 * (j % 4)
                        cs = (j // 4) * D
                        mv = xts[g][rs : rs + K, cs : cs + D]
                        wt0 = cb_t[rs : rs + K, 0:P0]
                        ps0 = psum.tile([P0, D], F32, tag="ps0", bufs=4)
                        nc.tensor.matmul(
                            ps0, wt0, mv, start=True, stop=True, tile_position=(rs, 0)
                        )
                        nc.vector.tensor_copy(st0[:, jj * D : (jj + 1) * D], ps0)

                    # phase B: transposed tail, interleaved across PAIRS of
                    # batches so consecutive matmuls alternate both PE strip
                    # and PSUM bank (weight loads overlap the prior matmul),
                    # while same-bank writes stay within one strip (ordered)
                    for jj0 in range(0, sub, 2):
                        pa = psum.tile([P0, 4 * P1], F32, tag="ps1", bufs=4)
                        pb = psum.tile([P0, 4 * P1], F32, tag="ps1", bufs=4)
                        for q in range(4):
                            for k, pt in ((0, pa), (1, pb)):
                                j = sub0 + jj0 + k
                                rs = 32 * (j % 4)
                                cs = (j // 4) * D
                                nc.tensor.matmul(
                                    pt[:, q * P1 : (q + 1) * P1],
                                    xts[g][
                                        rs : rs + K, cs + 128 * q : cs + 128 * (q + 1)
                                    ],
                                    cb_t[rs : rs + K, P0:T],
                                    start=True,
                                    stop=True,
                                    tile_position=(rs, 0),
                                )
                        for k, pt in ((0, pa), (1, pb)):
                            jj = jj0 + k
                            nc.scalar.copy(
                                st1[:, jj * 4 * P1 : (jj + 1) * 4 * P1], pt
                            )

                    # alternate the two HWDGE rings (SP / Activation) per store
                    # so bandwidth and completion latency overlap
                    bs = b0 + sub0
                    eng0, eng1 = (
                        (nc.sync, nc.scalar) if nstore % 2 == 0 else (nc.scalar, nc.sync)
                    )
                    eng0.dma_start(out=out[:, bs * D : (bs + sub) * D], in_=st0)
                    eng1.dma_start(
                        out=outt[:, bs * 4 * P1 : (bs + sub) * 4 * P1], in_=st1
                    )
                    nstore += 1
                    sub0 += sub

    nc.finalize()
    return nc


def _unroll_coeffs(W: np.ndarray, b: np.ndarray) -> np.ndarray:
    """Unroll the linear AR recurrence: CB[k, t] with rows 0..23 = window
    coefficients, row 24 = additive bias per step."""
    w = W[:, 0].astype(np.float64)
    bb = float(np.asarray(b).reshape(-1)[0])
    M = np.eye(ORDER)
    m = np.zeros(ORDER)
    CB = np.zeros((K, T), np.float64)
    for t in range(T):
        c = M.T @ w
        yb = m @ w + bb
        CB[:ORDER, t] = c
        CB[ORDER, t] = yb
        M = np.vstack([M[1:], c[None, :]])
        m = np.concatenate([m[1:], [yb]])
    return CB.astype(np.float32)


def _pack_inputs(x: np.ndarray) -> np.ndarray:
    """Build per-core packed moving operands.

    Returns [N_CORES, 128, (NB//4)*D] where core c / block b (local batch)
    sits at row-slot b%4 (25 rows), col-slot b//4 (512 cols); block contents =
    [x[global_b, S-24+i, d] for i rows] plus a trailing ones row.
    """
    xw = x[:, -ORDER:, :]  # [B, 24, D]
    packed = np.zeros((N_CORES, 128, (NB // 4) * D), np.float32)
    for c in range(N_CORES):
        for b in range(NB):
            rs = 32 * (b % 4)
            cs = (b // 4) * D
            blk = xw[c * NB + b]  # [24, D]
            packed[c, rs : rs + ORDER, cs : cs + D] = blk
            packed[c, rs + ORDER, cs : cs + D] = 1.0
    return packed


def kernel(x, W, b, tar_seq_len):
    global _nc_cache
    x = np.asarray(x, dtype=np.float32)
    W = np.asarray(W, dtype=np.float32)
    b = np.asarray(b, dtype=np.float32)
    assert int(tar_seq_len) == T, f"compiled for tar_seq_len={T}"
    assert x.shape == (B, S, D)

    CB = _unroll_coeffs(W, b)
    packed = _pack_inputs(x)

    # replicate CB into each 32-row strip of the PE array (rows 25-31 zero)
    CBrep = np.zeros((128, T), np.float32)
    for s in range(4):
        CBrep[32 * s : 32 * s + K] = CB

    if _nc_cache is None:
        _nc_cache = _build_program()
    nc = _nc_cache

    in_maps = [{"xpack": packed[c], "cb": CBrep} for c in range(N_CORES)]
    res = run_bass_kernel_spmd(nc, in_maps, list(range(N_CORES)))

    # gather: chunk0 [128, NB*D] -> [NB, 128, D]; transposed tail
    # [128, (g, j, q, t')] -> [NB, 40, D] with d = 128*q + p
    P1 = T - 128
    parts = []
    for r in res.results:
        y = np.empty((NB, T, D), np.float32)
        y[:, 0:128, :] = r["out"].reshape(128, NB, D).transpose(1, 0, 2)
        tail = r["outt"].reshape(128, NB, 4, P1)
        y[:, 128:T, :] = tail.transpose(1, 3, 2, 0).reshape(NB, P1, D)
        parts.append(y)
    return np.ascontiguousarray(np.concatenate(parts, axis=0))


# revision 66
# speedup vs baseline: 1.0303x; 1.0303x over previous
"""AR(24) extrapolation kernel for Trainium2 (8 NeuronCores, data parallel).

The reference runs a 168-step scalar-weight autoregressive recurrence over the
last 24 timesteps of x, independently per (batch, channel).  Because the
recurrence is linear, output step t is a fixed linear combination of the
initial 24-sample window plus a bias term:

    y[b, t, d] = sum_i C[i, t] * x[b, S-24+i, d] + beta[t]

C [24, 168] and beta [168] follow from W/b by unrolling the recurrence once on
the host (float64, ~4k flops).  The device work is then a memory-bound
broadcast matmul: per core, out[t, (b, d)] = CB^T @ xaug where CB stacks
[C; beta] and xaug stacks [x_window^T; ones].

Sharding: pure data parallelism over batch (32 batches per core).

Layout details per core (all chosen so every DMA uses all 16 SBUF ports with
multi-KB contiguous runs, since the kernel is store-bandwidth-bound):
- input xpack [128, 4096]: 32 moving blocks of [25, 512] (24 window rows plus
  a ones row that carries the bias through the contraction); block b sits at
  row strip 32*(b%4) (PE operands must start on a 32-partition boundary) and
  columns (b//4)*512.
- weights cb [128, 168]: CB replicated into each 32-row strip.
- t chunk 0 (t 0..127): matmul stationary CB[:, :128] -> PSUM [128, 512] per
  batch; staged as [128, sz*512] and stored to out[128, 16384] ([t, b*D+d]).
- t tail (t 128..167): computed in transposed orientation (stationary =
  x d-chunk [25, 128], moving = CB[:, 128:168]) so the result lands as
  [d%128, t'] on all 128 partitions; staged [128, sz*160] and stored to
  outt[128, NB*4*40]; the host untransposes while gathering.
- float32r operands: fast fp32 matmul path on the PE (4x fp32 rate; ~1e-4
  relative error from TF32-style operand rounding).
- stores alternate between the two HWDGE rings (sync / scalar engines);
  group sizes [4, 4, 8, 8, 8] start the store stream early, which paces the
  whole kernel (~420 GB/s sustained, near the 435 GB/s SBUF fabric ceiling).
"""

import numpy as np

import concourse.bacc as bacc
import concourse.tile as tile
from concourse import mybir
from concourse.bass_utils import run_bass_kernel_spmd

ORDER = 24
K = ORDER + 1            # contraction: 24 window rows + ones row
T = 168
D = 512
B = 256
S = 336
N_CORES = 8
NB = B // N_CORES        # 32 local batches per core
COLS = NB * D            # 16384 columns per core
GROUPS = [4, 4, 8, 8, 8]  # batches per staged output group (small first so
                          # the store stream starts early, then steady-state)
assert sum(GROUPS) == NB
# store-chunk (bs, sub) list: first group stores in 2-batch chunks so the
# store stream starts as early as possible
SUBS = []
_b = 0
for _g, _sz in enumerate(GROUPS):
    for _s in ([2, 2] if _g == 0 else [_sz]):
        SUBS.append((_b, _s))
        _b += _s
F32 = mybir.dt.float32
F32R = mybir.dt.float32r  # fast fp32 matmul path (full PE rate at N>=256)

_nc_cache = None


def _build_program():
    nc = bacc.Bacc()
    xp = nc.declare_dram_parameter("xpack", [128, (NB // 4) * D], F32R, isOutput=False)
    cb = nc.declare_dram_parameter("cb", [128, T], F32R, isOutput=False)
    # chunk0: t 0..127 as [t, (b, d)]; tail: t 128..167 stored transposed as
    # [d%128, (g, j, d//128, t-128)] so its stores cover all 128 partitions
    out = nc.declare_dram_parameter("out", [128, COLS], F32, isOutput=True)
    outt = nc.declare_dram_parameter(
        "outt", [128, NB * 4 * (T - 128)], F32, isOutput=True
    )

    with tile.TileContext(nc) as tc:
        with (
            tc.tile_pool(name="consts", bufs=1) as consts,
            tc.tile_pool(name="xin", bufs=1) as xin,
            tc.tile_pool(name="stage", bufs=3) as stage,
            tc.tile_pool(name="psum", bufs=4, space="PSUM") as psum,
        ):
            # Early-needed input loads go on the HWDGE rings (idle until the
            # store stream starts): weights first (every matmul needs them),
            # and the first tile split by partition halves so batches 0-1
            # (rows 0-63) land before batches 2-3.  Late-group inputs ride
            # the gpsimd SWDGE ring, keeping the HWDGE rings clear so the
            # first stores are not queued behind input transfers.
            cb_t = consts.tile([128, T], F32R)
            nc.sync.dma_start(out=cb_t, in_=cb[:, :])

            # PE warm-up: ~4.5us of dummy matmuls on a zeroed tile while the
            # first input load is in flight.  The HAM clock gate needs ~3.4us
            # of sustained PE activity to unthrottle 1.2 -> 2.4 GHz; without
            # this, every real matmul runs at 2-4x its warm cost and the PE
            # stalls the store stream mid-kernel.
            warm = consts.tile([32, D], F32)
            nc.gpsimd.memset(warm, 0.0)
            wps = psum.tile([P0w := 128, D], F32, tag="ps0", bufs=4)
            for _ in range(10):
                nc.tensor.matmul(
                    wps,
                    warm[0:K, 0:128].bitcast(F32R),
                    warm[0:K, :].bitcast(F32R),
                    start=True,
                    stop=True,
                    tile_position=(0, 0),
                )

            starts = [sum(GROUPS[:g]) for g in range(len(GROUPS))]
            xts = []
            for g, (b0, sz) in enumerate(zip(starts, GROUPS)):
                xt = xin.tile([128, sz * 128], F32R, tag=f"xt{g}")
                src = xp[:, b0 * 128 : (b0 + sz) * 128]
                if g == 0:
                    nc.scalar.dma_start(out=xt[0:64, :], in_=src[0:64, :])
                    nc.scalar.dma_start(out=xt[64:128, :], in_=src[64:128, :])
                elif g == 1:
                    nc.sync.dma_start(out=xt, in_=src)
                else:
                    eng = nc.scalar if g % 2 == 0 else nc.sync
                    eng.dma_start(out=xt, in_=src)
                xts.append(xt)

            P0 = 128
            P1 = T - P0  # 40
            SPLITS = [[s for bs, s in SUBS if b0 <= bs < b0 + sz]
                      for b0, sz in zip(starts, GROUPS)]
            nstore = 0
            for g, (b0, sz) in enumerate(zip(starts, GROUPS)):
                sub0 = 0
                for sub in SPLITS[g]:
                    st0 = stage.tile([P0, sub * D], F32, tag="st0")
                    st1 = stage.tile([P0, sub * 4 * P1], F32, tag="st1")
                    # phase A: chunk0 matmuls (consecutive batches sit in
                    # different 32-row strips, so LDW overlaps the prior MM)
                    for jj in range(sub):
                        j = sub0 + jj
                        rs = 32 * (j % 4)
                        cs = (j // 4) * D
                        mv = xts[g][rs : rs + K, cs : cs + D]
                        wt0 = cb_t[rs : rs + K, 0:P0]
                        ps0 = psum.tile([P0, D], F32, tag="ps0", bufs=4)
                        nc.tensor.matmul(
                            ps0, wt0, mv, start=True, stop=True, tile_position=(rs, 0)
                        )
                        nc.vector.tensor_copy(st0[:, jj * D : (jj + 1) * D], ps0)

                    # phase B: transposed tail, interleaved across PAIRS of
                    # batches so consecutive matmuls alternate both PE strip
                    # and PSUM bank (weight loads overlap the prior matmul),
                    # while same-bank writes stay within one strip (ordered)
                    for jj0 in range(0, sub, 2):
                        pa = psum.tile([P0, 4 * P1], F32, tag="ps1", bufs=4)
                        pb = psum.tile([P0, 4 * P1], F32, tag="ps1", bufs=4)
                        for q in range(4):
                            for k, pt in ((0, pa), (1, pb)):
                                j = sub0 + jj0 + k
                                rs = 32 * (j % 4)
                                cs = (j // 4) * D
                                nc.tensor.matmul(
                                    pt[:, q * P1 : (q + 1) * P1],
                                    xts[g][
                                        rs : rs + K, cs + 128 * q : cs + 128 * (q + 1)
                                    ],
                                    cb_t[rs : rs + K, P0:T],
                                    start=True,
                                    stop=True,
                                    tile_position=(rs, 0),
                                )
                        for k, pt in ((0, pa), (1, pb)):
                            jj = jj0 + k
                            nc.scalar.copy(
                                st1[:, jj * 4 * P1 : (jj + 1) * 4 * P1], pt
                            )

                    # alternate the two HWDGE rings (SP / Activation) per store
                    # so bandwidth and completion latency overlap
                    bs = b0 + sub0
                    eng0, eng1 = (
                        (nc.sync, nc.scalar) if nstore % 2 == 0 else (nc.scalar, nc.sync)
                    )
                    eng0.dma_start(out=out[:, bs * D : (bs + sub) * D], in_=st0)
                    eng1.dma_start(
                        out=outt[:, bs * 4 * P1 : (bs + sub) * 4 * P1], in_=st1
                    )
                    nstore += 1
                    sub0 += sub

    nc.finalize()
    return nc


def _unroll_coeffs(W: np.ndarray, b: np.ndarray) -> np.ndarray:
    """Unroll the linear AR recurrence: CB[k, t] with rows 0..23 = window
    coefficients, row 24 = additive bias per step."""
    w = W[:, 0].astype(np.float64)
    bb = float(np.asarray(b).reshape(-1)[0])
    M = np.eye(ORDER)
    m = np.zeros(ORDER)
    CB = np.zeros((K, T), np.float64)
    for t in range(T):
        c = M.T @ w
        yb = m @ w + bb
        CB[:ORDER, t] = c
        CB[ORDER, t] = yb
        M = np.vstack([M[1:], c[None, :]])
        m = np.concatenate([m[1:], [yb]])
    return CB.astype(np.float32)


def _pack_inputs(x: np.ndarray) -> np.ndarray:
    """Build per-core packed moving operands.

    Returns [N_CORES, 128, (NB//4)*D] where core c / block b (local batch)
    sits at row-slot b%4 (25 rows), col-slot b//4 (512 cols); block contents =
    [x[global_b, S-24+i, d] for i rows] plus a trailing ones row.
    """
    xw = x[:, -ORDER:, :]  # [B, 24, D]
    packed = np.zeros((N_CORES, 128, (NB // 4) * D), np.float32)
    for c in range(N_CORES):
        for b in range(NB):
            rs = 32 * (b % 4)
            cs = (b // 4) * D
            blk = xw[c * NB + b]  # [24, D]
            packed[c, rs : rs + ORDER, cs : cs + D] = blk
            packed[c, rs + ORDER, cs : cs + D] = 1.0
    return packed


def kernel(x, W, b, tar_seq_len):
    global _nc_cache
    x = np.asarray(x, dtype=np.float32)
    W = np.asarray(W, dtype=np.float32)
    b = np.asarray(b, dtype=np.float32)
    assert int(tar_seq_len) == T, f"compiled for tar_seq_len={T}"
    assert x.shape == (B, S, D)

    CB = _unroll_coeffs(W, b)
    packed = _pack_inputs(x)

    # replicate CB into each 32-row strip of the PE array (rows 25-31 zero)
    CBrep = np.zeros((128, T), np.float32)
    for s in range(4):
        CBrep[32 * s : 32 * s + K] = CB

    if _nc_cache is None:
        _nc_cache = _build_program()
    nc = _nc_cache

    in_maps = [{"xpack": packed[c], "cb": CBrep} for c in range(N_CORES)]
    res = run_bass_kernel_spmd(nc, in_maps, list(range(N_CORES)))

    # gather: chunk0 [128, NB*D] -> [NB, 128, D]; transposed tail
    # [128, (g, j, q, t')] -> [NB, 40, D] with d = 128*q + p
    P1 = T - 128
    parts = []
    for r in res.results:
        y = np.empty((NB, T, D), np.float32)
        y[:, 0:128, :] = r["out"].reshape(128, NB, D).transpose(1, 0, 2)
        tail = r["outt"].reshape(128, NB, 4, P1)
        y[:, 128:T, :] = tail.transpose(1, 3, 2, 0).reshape(NB, P1, D)
        parts.append(y)
    return np.ascontiguousarray(np.concatenate(parts, axis=0))


# revision 68
# speedup vs baseline: 1.1179x; 1.0850x over previous
"""AR(24) extrapolation kernel for Trainium2 (8 NeuronCores, data parallel).

The reference runs a 168-step scalar-weight autoregressive recurrence over the
last 24 timesteps of x, independently per (batch, channel).  Because the
recurrence is linear, output step t is a fixed linear combination of the
initial 24-sample window plus a bias term:

    y[b, t, d] = sum_i C[i, t] * x[b, S-24+i, d] + beta[t]

C [24, 168] and beta [168] follow from W/b by unrolling the recurrence once on
the host (float64, ~4k flops).  The device work is then a memory-bound
broadcast matmul: per core, out[t, (b, d)] = CB^T @ xaug where CB stacks
[C; beta] and xaug stacks [x_window^T; ones].

Sharding: pure data parallelism over batch (32 batches per core).

Layout details per core (all chosen so every DMA uses all 16 SBUF ports with
multi-KB contiguous runs, since the kernel is store-bandwidth-bound):
- input xpack [128, 4096]: 32 moving blocks of [25, 512] (24 window rows plus
  a ones row that carries the bias through the contraction); block b sits at
  row strip 32*(b%4) (PE operands must start on a 32-partition boundary) and
  columns (b//4)*512.
- weights cb [128, 168]: CB replicated into each 32-row strip.
- t chunk 0 (t 0..127): matmul stationary CB[:, :128] -> PSUM [128, 512] per
  batch; staged as [128, sz*512] and stored to out[128, 16384] ([t, b*D+d]).
- t tail (t 128..167): computed in transposed orientation (stationary =
  x d-chunk [25, 128], moving = CB[:, 128:168]) so the result lands as
  [d%128, t'] on all 128 partitions; staged [128, sz*160] and stored to
  outt[128, NB*4*40]; the host untransposes while gathering.
- float32r operands: fast fp32 matmul path on the PE (4x fp32 rate; ~1e-4
  relative error from TF32-style operand rounding).
- stores alternate between the two HWDGE rings (sync / scalar engines);
  group sizes [4, 4, 8, 8, 8] start the store stream early, which paces the
  whole kernel (~420 GB/s sustained, near the 435 GB/s SBUF fabric ceiling).
"""

import numpy as np

import concourse.bacc as bacc
import concourse.tile as tile
from concourse import mybir
from concourse.bass_utils import run_bass_kernel_spmd

ORDER = 24
K = ORDER + 1            # contraction: 24 window rows + ones row
T = 168
D = 512
B = 256
S = 336
N_CORES = 8
NB = B // N_CORES        # 32 local batches per core
COLS = NB * D            # 16384 columns per core
GROUPS = [4, 4, 8, 8, 8]  # batches per staged output group (small first so
                          # the store stream starts early, then steady-state)
assert sum(GROUPS) == NB
# store-chunk (bs, sub) list: first group stores in 2-batch chunks so the
# store stream starts as early as possible
SUBS = []
_b = 0
for _g, _sz in enumerate(GROUPS):
    for _s in ([2, 2] if _g == 0 else [_sz]):
        SUBS.append((_b, _s))
        _b += _s
F32 = mybir.dt.float32
F16 = mybir.dt.float16   # full PE rate at any clock state + FWL fast weight loads

_nc_cache = None


def _build_program():
    nc = bacc.Bacc()
    xp = nc.declare_dram_parameter("xpack", [128, (NB // 4) * D], F16, isOutput=False)
    cb = nc.declare_dram_parameter("cb", [128, T], F16, isOutput=False)
    # chunk0: t 0..127 as [t, (b, d)]; tail: t 128..167 stored transposed as
    # [d%128, (g, j, d//128, t-128)] so its stores cover all 128 partitions
    out = nc.declare_dram_parameter("out", [128, COLS], F32, isOutput=True)
    outt = nc.declare_dram_parameter(
        "outt", [128, NB * 4 * (T - 128)], F32, isOutput=True
    )

    with tile.TileContext(nc) as tc:
        with (
            tc.tile_pool(name="consts", bufs=1) as consts,
            tc.tile_pool(name="xin", bufs=1) as xin,
            tc.tile_pool(name="stage", bufs=3) as stage,
            tc.tile_pool(name="psum", bufs=4, space="PSUM") as psum,
        ):
            # Early-needed input loads go on the HWDGE rings (idle until the
            # store stream starts): weights first (every matmul needs them),
            # and the first tile split by partition halves so batches 0-1
            # (rows 0-63) land before batches 2-3.  Late-group inputs ride
            # the gpsimd SWDGE ring, keeping the HWDGE rings clear so the
            # first stores are not queued behind input transfers.
            cb_t = consts.tile([128, T], F16)
            nc.sync.dma_start(out=cb_t, in_=cb[:, :])

            starts = [sum(GROUPS[:g]) for g in range(len(GROUPS))]
            xts = []
            for g, (b0, sz) in enumerate(zip(starts, GROUPS)):
                xt = xin.tile([128, sz * 128], F16, tag=f"xt{g}")
                src = xp[:, b0 * 128 : (b0 + sz) * 128]
                if g == 0:
                    nc.scalar.dma_start(out=xt[0:64, :], in_=src[0:64, :])
                    nc.scalar.dma_start(out=xt[64:128, :], in_=src[64:128, :])
                elif g == 1:
                    nc.sync.dma_start(out=xt, in_=src)
                else:
                    eng = nc.scalar if g % 2 == 0 else nc.sync
                    eng.dma_start(out=xt, in_=src)
                xts.append(xt)

            P0 = 128
            P1 = T - P0  # 40
            SPLITS = [[s for bs, s in SUBS if b0 <= bs < b0 + sz]
                      for b0, sz in zip(starts, GROUPS)]
            nstore = 0
            for g, (b0, sz) in enumerate(zip(starts, GROUPS)):
                sub0 = 0
                for sub in SPLITS[g]:
                    st0 = stage.tile([P0, sub * D], F32, tag="st0")
                    st1 = stage.tile([P0, sub * 4 * P1], F32, tag="st1")
                    # phase A: chunk0 matmuls (consecutive batches sit in
                    # different 32-row strips, so LDW overlaps the prior MM)
                    for jj in range(sub):
                        j = sub0 + jj
                        rs = 32 * (j % 4)
                        cs = (j // 4) * D
                        mv = xts[g][rs : rs + K, cs : cs + D]
                        wt0 = cb_t[rs : rs + K, 0:P0]
                        ps0 = psum.tile([P0, D], F32, tag="ps0", bufs=4)
                        nc.tensor.matmul(
                            ps0, wt0, mv, start=True, stop=True, tile_position=(rs, 0)
                        )
                        nc.vector.tensor_copy(st0[:, jj * D : (jj + 1) * D], ps0)

                    # phase B: transposed tail, interleaved across PAIRS of
                    # batches so consecutive matmuls alternate both PE strip
                    # and PSUM bank (weight loads overlap the prior matmul),
                    # while same-bank writes stay within one strip (ordered)
                    for jj0 in range(0, sub, 2):
                        pa = psum.tile([P0, 4 * P1], F32, tag="ps1", bufs=4)
                        pb = psum.tile([P0, 4 * P1], F32, tag="ps1", bufs=4)
                        for q in range(4):
                            for k, pt in ((0, pa), (1, pb)):
                                j = sub0 + jj0 + k
                                rs = 32 * (j % 4)
                                cs = (j // 4) * D
                                nc.tensor.matmul(
                                    pt[:, q * P1 : (q + 1) * P1],
                                    xts[g][
                                        rs : rs + K, cs + 128 * q : cs + 128 * (q + 1)
                                    ],
                                    cb_t[rs : rs + K, P0:T],
                                    start=True,
                                    stop=True,
                                    tile_position=(rs, 0),
                                )
                        for k, pt in ((0, pa), (1, pb)):
                            jj = jj0 + k
                            nc.scalar.copy(
                                st1[:, jj * 4 * P1 : (jj + 1) * 4 * P1], pt
                            )

                    # alternate the two HWDGE rings (SP / Activation) per store
                    # so bandwidth and completion latency overlap
                    bs = b0 + sub0
                    eng0, eng1 = (
                        (nc.sync, nc.scalar) if nstore % 2 == 0 else (nc.scalar, nc.sync)
                    )
                    eng0.dma_start(out=out[:, bs * D : (bs + sub) * D], in_=st0)
                    eng1.dma_start(
                        out=outt[:, bs * 4 * P1 : (bs + sub) * 4 * P1], in_=st1
                    )
                    nstore += 1
                    sub0 += sub

    nc.finalize()
    return nc


def _unroll_coeffs(W: np.ndarray, b: np.ndarray) -> np.ndarray:
    """Unroll the linear AR recurrence: CB[k, t] with rows 0..23 = window
    coefficients, row 24 = additive bias per step."""
    w = W[:, 0].astype(np.float64)
    bb = float(np.asarray(b).reshape(-1)[0])
    M = np.eye(ORDER)
    m = np.zeros(ORDER)
    CB = np.zeros((K, T), np.float64)
    for t in range(T):
        c = M.T @ w
        yb = m @ w + bb
        CB[:ORDER, t] = c
        CB[ORDER, t] = yb
        M = np.vstack([M[1:], c[None, :]])
        m = np.concatenate([m[1:], [yb]])
    return CB.astype(np.float32)


def _pack_inputs(x: np.ndarray) -> np.ndarray:
    """Build per-core packed moving operands.

    Returns [N_CORES, 128, (NB//4)*D] where core c / block b (local batch)
    sits at row-slot b%4 (25 rows), col-slot b//4 (512 cols); block contents =
    [x[global_b, S-24+i, d] for i rows] plus a trailing ones row.
    """
    xw = x[:, -ORDER:, :]  # [B, 24, D]
    packed = np.zeros((N_CORES, 128, (NB // 4) * D), np.float32)
    for c in range(N_CORES):
        for b in range(NB):
            rs = 32 * (b % 4)
            cs = (b // 4) * D
            blk = xw[c * NB + b]  # [24, D]
            packed[c, rs : rs + ORDER, cs : cs + D] = blk
            packed[c, rs + ORDER, cs : cs + D] = 1.0
    return packed


def _make_in_maps(x, W, b):
    CB = _unroll_coeffs(W, b)
    packed = _pack_inputs(x)

    # fp16 operands with a folded power-of-2 balance scale: the device
    # computes (CB/s)^T @ (x*s) whose products equal CB^T @ x exactly, while
    # both operands stay well inside fp16 range (geometric-mean split)
    maxc = max(float(np.abs(CB).max()), 1e-30)
    maxx = max(float(np.abs(packed).max()), 1.0)
    s = 2.0 ** round((np.log2(maxc) - np.log2(maxx)) / 2.0)
    assert maxc / s < 6.0e4 and maxx * s < 6.0e4, "fp16 range exceeded"

    # replicate scaled CB into each 32-row strip of the PE array
    CBrep = np.zeros((128, T), np.float16)
    for st in range(4):
        CBrep[32 * st : 32 * st + K] = (CB / s).astype(np.float16)
    packed16 = (packed * s).astype(np.float16)

    return [{"xpack": packed16[c], "cb": CBrep} for c in range(N_CORES)]


def kernel(x, W, b, tar_seq_len):
    global _nc_cache
    x = np.asarray(x, dtype=np.float32)
    W = np.asarray(W, dtype=np.float32)
    b = np.asarray(b, dtype=np.float32)
    assert int(tar_seq_len) == T, f"compiled for tar_seq_len={T}"
    assert x.shape == (B, S, D)

    in_maps = _make_in_maps(x, W, b)

    if _nc_cache is None:
        _nc_cache = _build_program()
    nc = _nc_cache
    res = run_bass_kernel_spmd(nc, in_maps, list(range(N_CORES)))

    # gather: chunk0 [128, NB*D] -> [NB, 128, D]; transposed tail
    # [128, (g, j, q, t')] -> [NB, 40, D] with d = 128*q + p
    P1 = T - 128
    parts = []
    for r in res.results:
        y = np.empty((NB, T, D), np.float32)
        y[:, 0:128, :] = r["out"].reshape(128, NB, D).transpose(1, 0, 2)
        tail = r["outt"].reshape(128, NB, 4, P1)
        y[:, 128:T, :] = tail.transpose(1, 3, 2, 0).reshape(NB, P1, D)
        parts.append(y)
    return np.ascontiguousarray(np.concatenate(parts, axis=0))
